# revision 31
# baseline (speedup 1.0000x reference)
"""Trainium2 kernel for nn_RandomizedPruningMasks (scatter + linear).

Computes: w_mod = weight.reshape(-1).at[flip_idx].set(values * 0.1);
          y = x @ w_mod.T            # [B, I] x [O, I] -> [B, O]

Strategy (8 NeuronCores, SPMD):
  - Shard weight along output dim O: core c owns rows [c*OS, (c+1)*OS).
  - The scatter is folded into the streamed weight on the host.
  - One interleaved stream tensor st[P, NI, OS+B]: per I-itile the
    weight slice wT[P, OS] then the x slice xT[P, B]; 1536B contiguous
    per partition per itile keeps DMA descriptors at full rate.
  - Everything streams in fp16 (gate is scale-relative absmax ~2e-2;
    f16 keeps it ~3e-4): per core 6.3MB in + 0.26MB out; DMA floor
    ~15.4us at the measured dual-queue 410GB/s vs f16 PE floor ~14us.
  - Segment schedule: single-itile dma_starts for the first 8 and the
    last 4 itiles, 2-itile segments between, alternating the two HWDGE
    rings (one queue alone caps at ~244GB/s; both sustain ~410GB/s).
    Single-itile head: the queue start order is random with ~1.4us
    skew, so fine head granularity keeps the PE's early itile
    deadlines satisfiable by the late ring.  Single-itile tail: the
    last matmuls gate on a 1-itile completion instead of idling behind
    a 2-itile transfer.  Per-ring landings are in-order, so the PE
    chase self-heals after any skew stall.
  - 2 matmuls of N=512 per itile (per-instruction stationary reload is
    NOT free: N=256 variants measured ~20% more PE busy).
  - PE warmup (KWARM dummy matmuls on a memset tile) burns the
    0.65/1.2GHz p-state ramp during the DGE/transfer lead-in.
  - Per-core y_c = [B, OS] f16; host concatenates along the output dim.
"""

import os

import numpy as np

import concourse.mybir as mybir
import concourse.tile as tile
from concourse import bacc
from concourse.bass_utils import run_bass_kernel_spmd

N_CORES = 8
P = 128
VALUE_SCALE = 0.1

SEGS = [int(s) for s in os.environ.get(
    'KSEGS', '1,1,1,1,1,1,1,1,2,2,2,2,2,2,2,2,2,2,1,1,1,1').split(',')]
# Warmup dummy matmuls: sized so the warmup train ends right as the
# first itile lands (~3us after program start).  Too few -> the PE
# idles before its first real matmul and the idle resets the p-state
# ramp (matmuls then run at ~1.2-1.3GHz instead of 2.4GHz for the next
# ~3us); too many -> real work starts late for no benefit.
KWARM = int(os.environ.get('KWARM', '22'))

TRACE = False
_TRACE_KW = {}

DT = mybir.dt.float16


def _build_program(O, I, B):
    OS = O // N_CORES
    NI = I // P
    n_btiles = B // P
    C = OS + B
    assert B % P == 0 and I % P == 0 and sum(SEGS) == NI
    bounds = np.concatenate([[0], np.cumsum(SEGS)]).astype(int)

    nc = bacc.Bacc("TRN2", target_bir_lowering=False, debug=False,
                   num_devices=N_CORES)

    st = nc.declare_dram_parameter("st", [P, NI * C], DT, isOutput=False)
    y = nc.declare_dram_parameter("y", [B, OS], DT, isOutput=True)

    with tile.TileContext(nc) as tc:
        with (
            tc.tile_pool(name="stp", bufs=1) as stp,
            tc.tile_pool(name="wp", bufs=1) as wp,
            tc.tile_pool(name="yp", bufs=1) as yp,
            tc.tile_pool(name="psum", bufs=1, space="PSUM") as psp,
        ):
            t_s = stp.tile([P, NI, C], DT, tag="st")
            t_ps = [psp.tile([P, OS], mybir.dt.float32, tag=f"ps{j}",
                             name=f"ps{j}")
                    for j in range(n_btiles)]

            st_v = st[:].rearrange("p (n c) -> p n c", c=C)
            rings = [nc.sync, nc.scalar]
            # One of the two queues starts ~1.4-2us after the other
            # (random which).  Segments alternate rings; within a ring
            # itiles are increasing, so landings are in-order per ring
            # and the PE chase self-heals after a skew stall.
            ring_sel = os.environ.get('KRING', 'pair')
            nseg = len(SEGS)
            for g in range(nseg):
                k0, k1 = int(bounds[g]), int(bounds[g + 1])
                if ring_sel == 'headA':
                    # all head singles on ring0: during the PE's
                    # p-state ramp one queue's ~244GB/s suffices, so
                    # the head has no cross-ring gating at all; ring1
                    # preloads mid-stream segments meanwhile.  Ring0
                    # also takes four middle pairs to balance totals
                    # 16/16; tail singles go to ring1.
                    if SEGS[g] == 1:
                        r = rings[0 if k0 < NI // 2 else 1]
                    else:
                        r = rings[0 if g in (10, 12, 14, 16) else 1]
                elif ring_sel == 'pair' and SEGS[g] == 1:
                    # pair consecutive head/tail singles on one ring:
                    # the late-starting queue then owns itile 2k/2k+1
                    # instead of every other itile, buying its first
                    # delivery ~0.85us more deadline slack
                    r = rings[(g // 2) % 2]
                else:
                    r = rings[g % 2]
                r.dma_start(out=t_s[:, k0:k1, :],
                            in_=st_v[:, k0:k1, :])

            if KWARM:
                # p-state warmup: short dummy matmuls with no DMA dep
                # keep the PE continuously busy from program start, so
                # the 0.65/1.2GHz ramp elapses before real data lands.
                t_wm = wp.tile([P, P], DT, tag="wm")
                nc.vector.memset(t_wm[:], 0.0)
                for _ in range(KWARM):
                    nc.tensor.matmul(out=t_ps[0][:, 0:P], lhsT=t_wm[:],
                                     rhs=t_wm[:], start=True, stop=True)

            for it in range(NI):
                for j in range(n_btiles):
                    nc.tensor.matmul(
                        out=t_ps[j][:],
                        lhsT=t_s[:, it, OS + j * P:OS + (j + 1) * P],
                        rhs=t_s[:, it, 0:OS],
                        start=(it == 0),
                        stop=(it == NI - 1),
                    )

            # epilogue on DVE only (Act would pull a 1.3us
            # ACT_TABLE_LOAD into the scalar queue at stream start);
            # DMA cannot read PSUM, so cast PSUM->SBUF f16 then store
            for j in range(n_btiles):
                t_y = yp.tile([P, OS], DT, tag=f"y{j}", name=f"y{j}")
                nc.vector.tensor_copy(t_y[:], t_ps[j][:])
                rings[j % 2].dma_start(out=y[j * P:(j + 1) * P, :],
                                       in_=t_y[:])

    nc.compile()
    return nc


def _prep_inputs(x, weight, flip_idx, values):
    """Host-side sharding: per-core [P, NI, OS+B] (wT|xT) stream."""
    O, I = weight.shape
    B = x.shape[0]
    OS = O // N_CORES
    NI = I // P
    np_dt = mybir.dt.np(DT)

    # apply the scatter on host in f32 (last write wins, matching the
    # reference's .at[].set), then round once to the stream dtype
    wf = weight.astype(np.float32).reshape(-1).copy()
    wf[np.asarray(flip_idx)] = (np.asarray(values, np.float32)
                                * np.float32(VALUE_SCALE))
    w_mod = wf.reshape(O, I)

    # xT tile layout: [it, p, b] = x[b, it*P + p]
    xt = x.T.astype(np.float32).reshape(NI, P, B)

    in_maps = []
    for ci in range(N_CORES):
        wT = w_mod[ci * OS:(ci + 1) * OS].T.reshape(NI, P, OS)
        stream = np.concatenate([wT, xt], axis=2)       # [NI, P, OS+B]
        stream = np.ascontiguousarray(
            stream.transpose(1, 0, 2)).reshape(P, NI * (OS + B))
        in_maps.append({"st": stream.astype(np_dt)})

    return in_maps, (O, I, B)


def kernel(x, weight, flip_idx, values):
    x = np.asarray(x)
    weight = np.asarray(weight)
    in_maps, (O, I, B) = _prep_inputs(x, weight, flip_idx, values)
    nc = _build_program(O, I, B)
    res = run_bass_kernel_spmd(nc, in_maps, list(range(N_CORES)),
                               trace=TRACE, **_TRACE_KW)
    if TRACE:
        kernel.last_result = res
    y = np.concatenate([np.asarray(res.results[c]["y"], dtype=np.float32)
                        for c in range(N_CORES)], axis=1)
    return y.astype(np.float32)


# revision 35
# speedup vs baseline: 1.1274x; 1.1274x over previous
"""Trainium2 kernel for nn_RandomizedPruningMasks (scatter + linear).

Computes: w_mod = weight.reshape(-1).at[flip_idx].set(values * 0.1);
          y = x @ w_mod.T            # [B, I] x [O, I] -> [B, O]

Strategy (8 NeuronCores, SPMD), 4-way output x 2-way contraction shard:
  - Core c = (oq, ih) owns w block [oq*1024:(oq+1)*1024, ih*2048:...]
    and the matching x half; it emits an f16 partial y [B, 1024]; the
    host sums the ih=0/1 partials per output quarter.  Queue bytes per
    core = w 4.19MB (fixed by any sharding) + x 2.1/k_i + y 2.1/k_o MB,
    so (k_o=4, k_i=2) moves 5.76MB vs 6.55MB for pure output sharding.
  - The scatter is folded into the streamed weight on the host.
  - One interleaved stream tensor st[P, NI, OS+B]: per I-itile the
    weight slice wT[P, 1024] then the x slice xT[P, 256]; 2.5KB
    contiguous per partition per itile keeps DMA descriptors at full
    rate.  f16 everywhere (gate is scale-relative absmax ~2e-2; this
    kernel measures ~4.5e-4).
  - Segment schedule: single-itile dma_starts at head and tail, 2-itile
    segments between, alternating two HWDGE rings (one queue alone caps
    at ~244GB/s; both sustain ~410GB/s).  The queue start order is
    random with ~1.4-2us skew, so consecutive singles pair per ring and
    per-ring landings stay in-order — the PE chase self-heals.
  - Per itile: 4 matmuls of N=512 (2 btiles x 2 output halves, each
    into its own PSUM bank — a matmul writing at a column offset
    inside a bank corrupts accumulation).  PE demand ~864ns/itile vs
    DMA delivery ~800ns/itile, so the warm PE never starves.
  - PE warmup (KWARM dummy matmuls on a memset tile) burns the
    0.65/1.2GHz p-state ramp during the DGE/transfer lead-in; an idle
    PE gap resets the ramp and pins matmuls at ~1.2-1.3GHz for ~3us.
"""

import os

import numpy as np

import concourse.mybir as mybir
import concourse.tile as tile
from concourse import bacc
from concourse.bass_utils import run_bass_kernel_spmd

N_CORES = 8
KO = 4                       # output-dim shards
KI = 2                       # contraction-dim shards
P = 128
VALUE_SCALE = 0.1

SEGS = [int(s) for s in os.environ.get(
    'KSEGS', '1,1,1,1,2,2,2,2,2,1,1').split(',')]
# Warmup dummy matmuls: sized so the warmup train ends right as the
# first itile lands (~3us after program start).  Too few -> the PE
# idles before its first real matmul and the idle resets the p-state
# ramp; too many -> real work starts late for no benefit.
KWARM = int(os.environ.get('KWARM', '26'))

TRACE = False
_TRACE_KW = {}

DT = mybir.dt.float16


def _build_program(O, I, B):
    OS = O // KO                 # 1024 output cols per core
    IS = I // KI                 # 2048 contraction cols per core
    NI = IS // P                 # 16 itiles
    n_btiles = B // P
    WH = 512                     # PSUM-bank-sized output half
    C = OS + B
    assert B % P == 0 and IS % P == 0 and sum(SEGS) == NI
    assert OS == 2 * WH
    bounds = np.concatenate([[0], np.cumsum(SEGS)]).astype(int)

    nc = bacc.Bacc("TRN2", target_bir_lowering=False, debug=False,
                   num_devices=N_CORES)

    st = nc.declare_dram_parameter("st", [P, NI * C], DT, isOutput=False)
    y = nc.declare_dram_parameter("y", [B, OS], DT, isOutput=True)

    with tile.TileContext(nc) as tc:
        with (
            tc.tile_pool(name="stp", bufs=1) as stp,
            tc.tile_pool(name="wp", bufs=1) as wp,
            tc.tile_pool(name="yp", bufs=1) as yp,
            tc.tile_pool(name="psum", bufs=1, space="PSUM") as psp,
        ):
            t_s = stp.tile([P, NI, C], DT, tag="st")
            t_ps = [[psp.tile([P, WH], mybir.dt.float32, tag=f"ps{j}{h}",
                              name=f"ps{j}{h}")
                     for h in range(2)] for j in range(n_btiles)]

            st_v = st[:].rearrange("p (n c) -> p n c", c=C)
            rings = [nc.sync, nc.scalar]
            # One of the two queues starts ~1.4-2us after the other
            # (random which).  Segments alternate rings; within a ring
            # itiles are increasing, so landings are in-order per ring
            # and the PE chase self-heals after a skew stall.
            ring_sel = os.environ.get('KRING', 'pair')
            for g in range(len(SEGS)):
                k0, k1 = int(bounds[g]), int(bounds[g + 1])
                if ring_sel == 'pair' and SEGS[g] == 1:
                    # pair consecutive head/tail singles on one ring:
                    # the late-starting queue then owns itile 2k/2k+1
                    # instead of every other itile, buying its first
                    # delivery more deadline slack
                    r = rings[(g // 2) % 2]
                else:
                    r = rings[g % 2]
                r.dma_start(out=t_s[:, k0:k1, :],
                            in_=st_v[:, k0:k1, :])

            if KWARM:
                # p-state warmup: short dummy matmuls with no DMA dep
                # keep the PE continuously busy from program start, so
                # the 0.65/1.2GHz ramp elapses before real data lands.
                t_wm = wp.tile([P, P], DT, tag="wm")
                nc.vector.memset(t_wm[:], 0.0)
                for _ in range(KWARM):
                    nc.tensor.matmul(out=t_ps[0][0][:, 0:P], lhsT=t_wm[:],
                                     rhs=t_wm[:], start=True, stop=True)

            for it in range(NI):
                for j in range(n_btiles):
                    for h in range(2):
                        nc.tensor.matmul(
                            out=t_ps[j][h][:],
                            lhsT=t_s[:, it, OS + j * P:OS + (j + 1) * P],
                            rhs=t_s[:, it, h * WH:(h + 1) * WH],
                            start=(it == 0),
                            stop=(it == NI - 1),
                        )

            # epilogue: DMA cannot read PSUM, so cast PSUM->SBUF f16
            # then store.  The four casts split across DVE and Act so
            # each store's pair finishes in ~0.6us instead of 2.4us
            # serial on DVE.  Act costs a one-time 1.3us ACT_TABLE_LOAD
            # hoisted to the scalar queue's start, but the stream now
            # has ~3us of slack vs the PE, so it is absorbed.
            t_ys = [yp.tile([P, OS], DT, tag=f"y{j}", name=f"y{j}")
                    for j in range(n_btiles)]
            for j in range(n_btiles):
                nc.vector.tensor_copy(t_ys[j][:, 0:WH], t_ps[j][0][:])
                nc.scalar.copy(t_ys[j][:, WH:OS], t_ps[j][1][:])
                rings[j % 2].dma_start(out=y[j * P:(j + 1) * P, :],
                                       in_=t_ys[j][:])

    nc.compile()
    return nc


def _prep_inputs(x, weight, flip_idx, values):
    """Host-side sharding: per-core [P, NI, OS+B] (wT|xT) stream."""
    O, I = weight.shape
    B = x.shape[0]
    OS = O // KO
    IS = I // KI
    NI = IS // P
    np_dt = mybir.dt.np(DT)

    # apply the scatter on host in f32 (last write wins, matching the
    # reference's .at[].set), then round once to the stream dtype
    wf = weight.astype(np.float32).reshape(-1).copy()
    wf[np.asarray(flip_idx)] = (np.asarray(values, np.float32)
                                * np.float32(VALUE_SCALE))
    w_mod = wf.reshape(O, I)

    xT = x.T.astype(np.float32)                     # [I, B]

    in_maps = []
    for ci in range(N_CORES):
        oq, ih = ci // KI, ci % KI
        wT = (w_mod[oq * OS:(oq + 1) * OS, ih * IS:(ih + 1) * IS]
              .T.reshape(NI, P, OS))                # [NI, P, OS]
        xt = xT[ih * IS:(ih + 1) * IS].reshape(NI, P, B)
        stream = np.concatenate([wT, xt], axis=2)   # [NI, P, OS+B]
        stream = np.ascontiguousarray(
            stream.transpose(1, 0, 2)).reshape(P, NI * (OS + B))
        in_maps.append({"st": stream.astype(np_dt)})

    return in_maps, (O, I, B)


def kernel(x, weight, flip_idx, values):
    x = np.asarray(x)
    weight = np.asarray(weight)
    in_maps, (O, I, B) = _prep_inputs(x, weight, flip_idx, values)
    nc = _build_program(O, I, B)
    res = run_bass_kernel_spmd(nc, in_maps, list(range(N_CORES)),
                               trace=TRACE, **_TRACE_KW)
    if TRACE:
        kernel.last_result = res
    OS = O // KO
    y = np.zeros((B, O), np.float32)
    for ci in range(N_CORES):
        oq = ci // KI
        y[:, oq * OS:(oq + 1) * OS] += np.asarray(
            res.results[ci]["y"], dtype=np.float32)
    return y
